# revision 4
# baseline (speedup 1.0000x reference)
"""Self-contained Trainium2 Bass kernel for the EdgeNetwork GNN problem.

kernel(**inputs) takes FULL unsharded inputs, returns the FULL [100000, 32]
fp32 output.

Sharding: by destination-node range across 8 cores (no collectives).

Host prep routes each edge to the core owning its dst, sorts by dst, packs
edges into 128-edge tiles (<=32 distinct dst nodes per tile, whole nodes
only), gathers neighbor features x = node_attr[src] into the per-tile
layout, and duplicates ea/slot entries pairwise so every device-side AP is
16-bit packed.

Per-core device dataflow (per 128-edge tile):
  - Z[e,(k,j)] = ea[e,k]*x[e,j] built edge-major on DVE at 2x_1p speed
    (fp16), one op per 4-tile chunk
  - A[e,s] = (slot[e] == s) 0/1 mask, one packed DVE op per chunk
  - segment-sum via matmul: U_g = Z_g^T A (full 128-wide contraction,
    PSUM col-blocks)
  - out_tile[s,i] = sum_g U_g^T B_g (accumulating matmuls) -> node-major
  - chunk-level PSUM drains on Act; outputs written PACKED (contiguous
    DMA); host unscatters packed rows to node order afterward.
No indirect DMA, no collectives; PE/DVE/Act only.
"""

import os
import sys
from contextlib import ExitStack

import numpy as np

for _p in ("/opt/trn_rl_repo", "/root/.axon_site/_ro/trn_rl_repo"):
    if os.path.isdir(_p) and _p not in sys.path:
        sys.path.insert(0, _p)

import concourse.mybir as mybir
import concourse.tile as tile
from concourse import bacc
from concourse.bass_utils import run_bass_kernel_spmd

N_NODES = 100000
D = 32
KE = 16
NCORES = 8
NPC = N_NODES // NCORES
SENTINEL = 16384

TILE_E = 128          # edge slots per tile
TILE_S = 32           # node slots per tile
TPC = 4               # tiles per chunk
CPS = 8               # chunks per super
TPS = TPC * CPS       # tiles per super (32)

F32 = mybir.dt.float32
F16 = mybir.dt.float16


# ---------------------------------------------------------------- host prep

def _pack_core(d_loc, e_src, e_ea, KP):
    """Pack one core's edges (sorted by local dst) into tiles."""
    deg = np.bincount(d_loc, minlength=NPC)
    uniq = np.nonzero(deg)[0]
    degs = deg[uniq]
    cumdeg = np.concatenate([[0], np.cumsum(degs)])
    n_nodes = len(uniq)

    bounds = []
    i0 = 0
    while i0 < n_nodes:
        hi = np.searchsorted(cumdeg, cumdeg[i0] + TILE_E, side="right") - 1
        hi = min(hi, i0 + TILE_S, n_nodes)
        assert hi > i0, "node degree exceeds TILE_E"
        bounds.append((i0, hi))
        i0 = hi
    nt = len(bounds)
    lo_arr = np.array([b[0] for b in bounds])

    rank_tile = np.zeros(n_nodes, np.int64)
    rank_tile[lo_arr[1:]] = 1
    rank_tile = np.cumsum(rank_tile)
    rank_slot = np.arange(n_nodes) - lo_arr[rank_tile]

    e_rank = np.searchsorted(uniq, d_loc)
    e_tile = rank_tile[e_rank]
    e_part = np.arange(len(d_loc)) - cumdeg[lo_arr[e_tile]]
    assert e_part.max() < TILE_E

    idx = e_tile * TILE_E + e_part
    srcA = np.zeros(nt * TILE_E, np.int64)
    srcA[idx] = e_src
    slotA = np.zeros(nt * TILE_E, np.float32)
    slotA[idx] = rank_slot[e_rank]
    eaA = np.zeros((nt * TILE_E, KP), np.float32)
    eaA[idx] = e_ea
    # edge-validity mask (padding edges must contribute zero; their ea
    # rows are zero already which guarantees that)

    sidxA = np.full((nt, TILE_S), SENTINEL, np.int32)
    sidxA[rank_tile, rank_slot] = uniq

    return nt, srcA.reshape(nt, TILE_E), slotA.reshape(nt, TILE_E), \
        eaA.reshape(nt, TILE_E, KP), sidxA


def _prepare(node_attr, edge_attr, pair_indices, kernel, bias):
    dst = np.asarray(pair_indices[:, 0], dtype=np.int64)
    src = np.asarray(pair_indices[:, 1], dtype=np.int64)
    ea = np.asarray(edge_attr, dtype=np.float32)
    kern = np.asarray(kernel, dtype=np.float32)
    bias = np.asarray(bias, dtype=np.float32)

    use_bias = bool(np.any(bias != 0.0))
    KP = KE + 1 if use_bias else KE
    kern_eff = np.concatenate([kern, bias[None, :]], 0) if use_bias else kern

    # B[k*32+j, i] = kern_eff[k, i*32+j]
    B = np.ascontiguousarray(
        kern_eff.reshape(KP, D, D).transpose(0, 2, 1).reshape(KP * D, D)
    ).astype(np.float16)

    node16 = np.ascontiguousarray(node_attr, dtype=np.float16)

    packed = []
    max_nt = 0
    for c in range(NCORES):
        lo = c * NPC
        sel = np.nonzero((dst >= lo) & (dst < lo + NPC))[0]
        d_loc = dst[sel] - lo
        order = np.argsort(d_loc, kind="stable")
        sel = sel[order]
        nt, srcA, slotA, eaA, sidxA = _pack_core(
            d_loc[order], src[sel],
            np.concatenate([ea[sel], np.ones((len(sel), 1), np.float32)], 1)
            if use_bias else ea[sel], KP)
        packed.append((srcA, slotA, eaA, sidxA))
        max_nt = max(max_nt, nt)

    NSUP = (max_nt + TPS - 1) // TPS
    ntp = NSUP * TPS

    per_core = []
    sidx_all = []
    for c in range(NCORES):
        srcA, slotA, eaA, sidxA = packed[c]
        nt = srcA.shape[0]
        srcP = np.zeros((ntp, TILE_E), np.int64)
        srcP[:nt] = srcA
        slotP = np.zeros((ntp, TILE_E), np.float32)
        slotP[:nt] = slotA
        eaP = np.zeros((ntp, TILE_E, KP), np.float32)
        eaP[:nt] = eaA
        sidxP = np.full((ntp, TILE_S), SENTINEL, np.int32)
        sidxP[:nt] = sidxA
        sidx_all.append(sidxP)

        ea_dup = np.repeat(eaP, 2, axis=2).astype(np.float16)
        amask = (slotP[:, :, None] ==
                 np.arange(TILE_S, dtype=np.float32)[None, None, :]
                 ).astype(np.float16)               # [ntp, 128, TILE_S]
        x_gath = node16[srcP]                       # [ntp, 128, D]

        def sw(a):  # [ntp, 128, w] -> [NSUP, 128, TPS*w]
            a = a.reshape(NSUP, TPS, TILE_E, -1)
            return np.ascontiguousarray(
                np.moveaxis(a, 1, 2)).reshape(NSUP, TILE_E, -1)

        per_core.append(dict(
            ea_sw=sw(ea_dup),
            x_sw=sw(x_gath),
            a_sw=sw(amask),
            B=B,
        ))
    meta = dict(NSUP=NSUP, KP=KP, sidx=sidx_all)
    return per_core, meta


def _unscatter(pout_list, sidx_all, NSUP):
    """pout [NSUP, TILE_S, TPS*D] per core -> full [N_NODES, D]."""
    out = np.zeros((N_NODES, D), np.float32)
    for c in range(NCORES):
        pout = pout_list[c].reshape(NSUP, TILE_S, TPS, D)
        arr = np.moveaxis(pout, 2, 1).reshape(NSUP * TPS, TILE_S, D)
        sidx = sidx_all[c]
        mask = sidx != SENTINEL
        out[c * NPC + sidx[mask]] = arr[mask]
    return out


# ------------------------------------------------------------- bass program

def _groups(KP):
    total = KP * D
    out = []
    c = 0
    while c < total:
        w = min(128, total - c)
        out.append((c, w))
        c += w
    return out


def _build(NSUP, KP):
    W2 = 2 * KP           # ea_dup width per tile
    ZW = KP * D           # Z width per tile
    grp = _groups(KP)
    NG = len(grp)
    UW = NG * TILE_S      # U width per tile
    GT = 8                # tiles per z/drain group
    NGRP = TPS // GT      # groups per super

    nc = bacc.Bacc("TRN2", target_bir_lowering=False, debug=False)

    ea_d = nc.dram_tensor("ea_sw", [NSUP, 128, TPS * W2], F16,
                          kind="ExternalInput").ap()
    x_d = nc.dram_tensor("x_sw", [NSUP, 128, TPS * D], F16,
                         kind="ExternalInput").ap()
    a_d = nc.dram_tensor("a_sw", [NSUP, 128, TPS * TILE_S], F16,
                         kind="ExternalInput").ap()
    b_d = nc.dram_tensor("B", [KP * D, D], F16, kind="ExternalInput").ap()
    pout_d = nc.dram_tensor("pout", [NSUP, TILE_S, TPS * D], F32,
                            kind="ExternalOutput").ap()

    with tile.TileContext(nc) as tc, ExitStack() as ctx:
        cpool = ctx.enter_context(tc.tile_pool(name="const", bufs=1))
        spool = ctx.enter_context(tc.tile_pool(name="sup", bufs=2))
        zpool = ctx.enter_context(tc.tile_pool(name="z", bufs=3))
        upool = ctx.enter_context(tc.tile_pool(name="usb", bufs=2))
        pu_pool = ctx.enter_context(
            tc.tile_pool(name="pu", bufs=2, space="PSUM"))
        po_pool = ctx.enter_context(
            tc.tile_pool(name="po", bufs=2, space="PSUM"))

        b_sb = cpool.tile([128, NG * D], F16, tag="b")
        for g, (c0, w) in enumerate(grp):
            nc.sync.dma_start(b_sb[0:w, g * D:(g + 1) * D],
                              b_d[c0:c0 + w, :])

        H = TPS // 2          # half-super tiles
        for s in range(NSUP):
            ea_sup = spool.tile([128, TPS * W2], F16, tag="ea")
            x_sup = spool.tile([128, TPS * D], F16, tag="x")
            a_sup = spool.tile([128, TPS * TILE_S], F16, tag="a")
            if s == 0:
                # split first loads so the first z-op starts sooner
                nc.sync.dma_start(x_sup[:, :GT * D], x_d[s][:, :GT * D])
                nc.sync.dma_start(ea_sup[:, :GT * W2], ea_d[s][:, :GT * W2])
                nc.sync.dma_start(x_sup[:, GT * D:], x_d[s][:, GT * D:])
                nc.sync.dma_start(ea_sup[:, GT * W2:], ea_d[s][:, GT * W2:])
                nc.sync.dma_start(a_sup[:, :H * TILE_S],
                                  a_d[s][:, :H * TILE_S])
                nc.sync.dma_start(a_sup[:, H * TILE_S:],
                                  a_d[s][:, H * TILE_S:])
            else:
                nc.sync.dma_start(ea_sup[:], ea_d[s])
                nc.sync.dma_start(x_sup[:], x_d[s])
                nc.sync.dma_start(a_sup[:], a_d[s])

            out_sb = spool.tile([TILE_S, TPS * D], F32, tag="osup")

            for half in range(2):
                o_ps = po_pool.tile([TILE_S, H * D], F32, tag="ops")
                for gih in range(H // GT):
                    t0 = half * H + gih * GT

                    z_sb = zpool.tile([128, GT * ZW], F16, tag="z")

                    def zop(eng, ta, nt, zoff):
                        ea_b = ea_sup[:, ta * W2:(ta + nt) * W2] \
                            .rearrange("p (t k o r) -> p t k o r",
                                       o=1, r=2, k=KP) \
                            .to_broadcast([128, nt, KP, 16, 2])
                        x_b = x_sup[:, ta * D:(ta + nt) * D] \
                            .rearrange("p (t o j2 r) -> p t o j2 r",
                                       o=1, r=2, j2=16) \
                            .to_broadcast([128, nt, KP, 16, 2])
                        z_ap = z_sb[:, zoff * ZW:(zoff + nt) * ZW] \
                            .rearrange("p (t k j2 r) -> p t k j2 r",
                                       k=KP, j2=16, r=2)
                        eng.tensor_tensor(out=z_ap, in0=x_b, in1=ea_b,
                                          op=mybir.AluOpType.mult)

                    if gih == 1:
                        # offload tail tiles of the 2nd group to gpsimd
                        NP = 3
                        zop(nc.vector, t0, GT - NP, 0)
                        zop(nc.gpsimd, t0 + GT - NP, NP, GT - NP)
                    else:
                        zop(nc.vector, t0, GT, 0)

                    u_ps = pu_pool.tile([128, GT * UW], F32, tag="u")
                    for tt in range(GT):
                        for g, (c0, w) in enumerate(grp):
                            col = tt * UW + g * TILE_S
                            zc = tt * ZW + c0
                            nc.tensor.matmul(
                                out=u_ps[0:w, col:col + TILE_S],
                                lhsT=z_sb[:, zc:zc + w],
                                rhs=a_sup[:, (t0 + tt) * TILE_S:
                                          (t0 + tt + 1) * TILE_S],
                                start=True, stop=True)

                    u_sb = upool.tile([128, GT * UW], F16, tag="usb")
                    nc.scalar.copy(out=u_sb[:], in_=u_ps[:])

                    for tt in range(GT):
                        for g, (c0, w) in enumerate(grp):
                            col = tt * UW + g * TILE_S
                            oc = (gih * GT + tt) * D
                            nc.tensor.matmul(
                                out=o_ps[:, oc:oc + D],
                                lhsT=u_sb[0:w, col:col + TILE_S],
                                rhs=b_sb[0:w, g * D:(g + 1) * D],
                                start=(g == 0), stop=(g == NG - 1))

                nc.scalar.copy(
                    out=out_sb[:, half * H * D:(half + 1) * H * D],
                    in_=o_ps[:])
                nc.sync.dma_start(
                    pout_d[s][:, half * H * D:(half + 1) * H * D],
                    out_sb[:, half * H * D:(half + 1) * H * D])

    nc.compile()
    return nc


_CACHE = {}


def kernel(node_attr, edge_attr, pair_indices, kernel, bias):
    per_core, meta = _prepare(node_attr, edge_attr, pair_indices,
                              kernel, bias)
    key = (meta["NSUP"], meta["KP"])
    if key not in _CACHE:
        _CACHE[key] = _build(*key)
    nc = _CACHE[key]
    res = run_bass_kernel_spmd(nc, per_core, list(range(NCORES)))
    pout = [res.results[c]["pout"] for c in range(NCORES)]
    return _unscatter(pout, meta["sidx"], meta["NSUP"])


# revision 5
# speedup vs baseline: 1.1266x; 1.1266x over previous
"""Self-contained Trainium2 Bass kernel for the EdgeNetwork GNN problem.

kernel(**inputs) takes FULL unsharded inputs, returns the FULL [100000, 32]
fp32 output.

Sharding: by destination-node range across 8 cores (no collectives).

Host prep routes each edge to the core owning its dst, sorts by dst, packs
edges into 128-edge tiles (<=32 distinct dst nodes per tile, whole nodes
only), gathers neighbor features x = node_attr[src] into the per-tile
layout, and duplicates ea/slot entries pairwise so every device-side AP is
16-bit packed.

Per-core device dataflow (per 128-edge tile):
  - Z[e,(k,j)] = ea[e,k]*x[e,j] built edge-major on DVE at 2x_1p speed
    (fp16), one op per 4-tile chunk
  - A[e,s] = (slot[e] == s) 0/1 mask, one packed DVE op per chunk
  - segment-sum via matmul: U_g = Z_g^T A (full 128-wide contraction,
    PSUM col-blocks)
  - out_tile[s,i] = sum_g U_g^T B_g (accumulating matmuls) -> node-major
  - chunk-level PSUM drains on Act; outputs written PACKED (contiguous
    DMA); host unscatters packed rows to node order afterward.
No indirect DMA, no collectives; PE/DVE/Act only.
"""

import os
import sys
from contextlib import ExitStack

import numpy as np

for _p in ("/opt/trn_rl_repo", "/root/.axon_site/_ro/trn_rl_repo"):
    if os.path.isdir(_p) and _p not in sys.path:
        sys.path.insert(0, _p)

import concourse.mybir as mybir
import concourse.tile as tile
from concourse import bacc
from concourse.bass_utils import run_bass_kernel_spmd

N_NODES = 100000
D = 32
KE = 16
NCORES = 8
NPC = N_NODES // NCORES
SENTINEL = 16384

TILE_E = 128          # edge slots per tile
TILE_S = 32           # node slots per tile
TPC = 4               # tiles per chunk
CPS = 8               # chunks per super
TPS = TPC * CPS       # tiles per super (32)

F32 = mybir.dt.float32
F16 = mybir.dt.float16


# ---------------------------------------------------------------- host prep

def _pack_core(d_loc, e_src, e_ea, KP):
    """Pack one core's edges (sorted by local dst) into tiles."""
    deg = np.bincount(d_loc, minlength=NPC)
    uniq = np.nonzero(deg)[0]
    degs = deg[uniq]
    cumdeg = np.concatenate([[0], np.cumsum(degs)])
    n_nodes = len(uniq)

    bounds = []
    i0 = 0
    while i0 < n_nodes:
        hi = np.searchsorted(cumdeg, cumdeg[i0] + TILE_E, side="right") - 1
        hi = min(hi, i0 + TILE_S, n_nodes)
        assert hi > i0, "node degree exceeds TILE_E"
        bounds.append((i0, hi))
        i0 = hi
    nt = len(bounds)
    lo_arr = np.array([b[0] for b in bounds])

    rank_tile = np.zeros(n_nodes, np.int64)
    rank_tile[lo_arr[1:]] = 1
    rank_tile = np.cumsum(rank_tile)
    rank_slot = np.arange(n_nodes) - lo_arr[rank_tile]

    e_rank = np.searchsorted(uniq, d_loc)
    e_tile = rank_tile[e_rank]
    e_part = np.arange(len(d_loc)) - cumdeg[lo_arr[e_tile]]
    assert e_part.max() < TILE_E

    idx = e_tile * TILE_E + e_part
    srcA = np.zeros(nt * TILE_E, np.int64)
    srcA[idx] = e_src
    slotA = np.zeros(nt * TILE_E, np.float32)
    slotA[idx] = rank_slot[e_rank]
    eaA = np.zeros((nt * TILE_E, KP), np.float32)
    eaA[idx] = e_ea
    # edge-validity mask (padding edges must contribute zero; their ea
    # rows are zero already which guarantees that)

    sidxA = np.full((nt, TILE_S), SENTINEL, np.int32)
    sidxA[rank_tile, rank_slot] = uniq

    return nt, srcA.reshape(nt, TILE_E), slotA.reshape(nt, TILE_E), \
        eaA.reshape(nt, TILE_E, KP), sidxA


def _prepare(node_attr, edge_attr, pair_indices, kernel, bias):
    dst = np.asarray(pair_indices[:, 0], dtype=np.int64)
    src = np.asarray(pair_indices[:, 1], dtype=np.int64)
    ea = np.asarray(edge_attr, dtype=np.float32)
    kern = np.asarray(kernel, dtype=np.float32)
    bias = np.asarray(bias, dtype=np.float32)

    use_bias = bool(np.any(bias != 0.0))
    KP = KE + 1 if use_bias else KE
    kern_eff = np.concatenate([kern, bias[None, :]], 0) if use_bias else kern

    # B[k*32+j, i] = kern_eff[k, i*32+j]
    B = np.ascontiguousarray(
        kern_eff.reshape(KP, D, D).transpose(0, 2, 1).reshape(KP * D, D)
    ).astype(np.float16)

    node16 = np.ascontiguousarray(node_attr, dtype=np.float16)

    packed = []
    max_nt = 0
    for c in range(NCORES):
        lo = c * NPC
        sel = np.nonzero((dst >= lo) & (dst < lo + NPC))[0]
        d_loc = dst[sel] - lo
        order = np.argsort(d_loc, kind="stable")
        sel = sel[order]
        nt, srcA, slotA, eaA, sidxA = _pack_core(
            d_loc[order], src[sel],
            np.concatenate([ea[sel], np.ones((len(sel), 1), np.float32)], 1)
            if use_bias else ea[sel], KP)
        packed.append((srcA, slotA, eaA, sidxA))
        max_nt = max(max_nt, nt)

    NSUP = (max_nt + TPS - 1) // TPS
    ntp = NSUP * TPS

    per_core = []
    sidx_all = []
    for c in range(NCORES):
        srcA, slotA, eaA, sidxA = packed[c]
        nt = srcA.shape[0]
        srcP = np.zeros((ntp, TILE_E), np.int64)
        srcP[:nt] = srcA
        slotP = np.zeros((ntp, TILE_E), np.float32)
        slotP[:nt] = slotA
        eaP = np.zeros((ntp, TILE_E, KP), np.float32)
        eaP[:nt] = eaA
        sidxP = np.full((ntp, TILE_S), SENTINEL, np.int32)
        sidxP[:nt] = sidxA
        sidx_all.append(sidxP)

        ea_dup = np.repeat(eaP, 2, axis=2).astype(np.float16)
        amask = (slotP[:, :, None] ==
                 np.arange(TILE_S, dtype=np.float32)[None, None, :]
                 ).astype(np.float16)               # [ntp, 128, TILE_S]
        x_gath = node16[srcP]                       # [ntp, 128, D]

        def sw(a):  # [ntp, 128, w] -> [NSUP, 128, TPS*w]
            a = a.reshape(NSUP, TPS, TILE_E, -1)
            return np.ascontiguousarray(
                np.moveaxis(a, 1, 2)).reshape(NSUP, TILE_E, -1)

        comb = np.concatenate(
            [sw(ea_dup), sw(x_gath), sw(amask)], axis=2)
        per_core.append(dict(comb_sw=comb, B=B))
    meta = dict(NSUP=NSUP, KP=KP, sidx=sidx_all)
    return per_core, meta


def _unscatter(pout_list, sidx_all, NSUP):
    """pout [NSUP, TILE_S, TPS*D] per core -> full [N_NODES, D]."""
    out = np.zeros((N_NODES, D), np.float32)
    for c in range(NCORES):
        pout = pout_list[c].reshape(NSUP, TILE_S, TPS, D)
        arr = np.moveaxis(pout, 2, 1).reshape(NSUP * TPS, TILE_S, D)
        sidx = sidx_all[c]
        mask = sidx != SENTINEL
        out[c * NPC + sidx[mask]] = arr[mask]
    return out


# ------------------------------------------------------------- bass program

def _groups(KP):
    total = KP * D
    out = []
    c = 0
    while c < total:
        w = min(128, total - c)
        out.append((c, w))
        c += w
    return out


def _build(NSUP, KP):
    W2 = 2 * KP           # ea_dup width per tile
    ZW = KP * D           # Z width per tile
    grp = _groups(KP)
    NG = len(grp)
    UW = NG * TILE_S      # U width per tile
    GT = 8                # tiles per z/drain group
    NGRP = TPS // GT      # groups per super

    nc = bacc.Bacc("TRN2", target_bir_lowering=False, debug=False)

    CW = TPS * (W2 + D + TILE_S)
    comb_d = nc.dram_tensor("comb_sw", [NSUP, 128, CW], F16,
                            kind="ExternalInput").ap()
    b_d = nc.dram_tensor("B", [KP * D, D], F16, kind="ExternalInput").ap()
    pout_d = nc.dram_tensor("pout", [NSUP, TILE_S, TPS * D], F32,
                            kind="ExternalOutput").ap()

    with tile.TileContext(nc) as tc, ExitStack() as ctx:
        cpool = ctx.enter_context(tc.tile_pool(name="const", bufs=1))
        spool = ctx.enter_context(tc.tile_pool(name="sup", bufs=3))
        zpool = ctx.enter_context(tc.tile_pool(name="z", bufs=3))
        upool = ctx.enter_context(tc.tile_pool(name="usb", bufs=3))
        pu_pool = ctx.enter_context(
            tc.tile_pool(name="pu", bufs=2, space="PSUM"))
        po_pool = ctx.enter_context(
            tc.tile_pool(name="po", bufs=2, space="PSUM"))

        b_sb = cpool.tile([128, NG * D], F16, tag="b")
        b_loaded = [False]

        def load_b():
            if not b_loaded[0]:
                b_loaded[0] = True
                for g, (c0, w) in enumerate(grp):
                    nc.sync.dma_start(b_sb[0:w, g * D:(g + 1) * D],
                                      b_d[c0:c0 + w, :])

        H = TPS // 2          # half-super tiles
        for s in range(NSUP):
            comb_sb = spool.tile([128, CW], F16, tag="comb")
            if s == 0:
                # split the first load so the first z-op starts sooner:
                # ea+x for the first 8 tiles, then the rest
                c1 = GT * W2
                nc.sync.dma_start(comb_sb[:, :c1], comb_d[s][:, :c1])
                c2 = TPS * W2 + GT * D
                nc.sync.dma_start(comb_sb[:, TPS * W2:c2],
                                  comb_d[s][:, TPS * W2:c2])
                nc.sync.dma_start(comb_sb[:, c1:TPS * W2],
                                  comb_d[s][:, c1:TPS * W2])
                nc.sync.dma_start(comb_sb[:, c2:], comb_d[s][:, c2:])
            else:
                nc.sync.dma_start(comb_sb[:], comb_d[s])
            load_b()
            ea_sup = comb_sb[:, :TPS * W2]
            x_sup = comb_sb[:, TPS * W2:TPS * (W2 + D)]
            a_sup = comb_sb[:, TPS * (W2 + D):]

            out_sb = spool.tile([TILE_S, TPS * D], F32, tag="osup")

            for half in range(2):
                o_ps = po_pool.tile([TILE_S, H * D], F32, tag="ops")
                if s == NSUP - 1 and half == 1:
                    # finer grain at the pipeline tail
                    parts = [(half * H + i * 4, 4, 1) for i in range(H // 4)]
                else:
                    parts = [(half * H + i * GT, GT, 3)
                             for i in range(H // GT)]
                for t0, gt, npool in parts:

                    z_sb = zpool.tile([128, gt * ZW], F16, tag="z")

                    def zop(eng, ta, nt, zoff):
                        ea_b = ea_sup[:, ta * W2:(ta + nt) * W2] \
                            .rearrange("p (t k o r) -> p t k o r",
                                       o=1, r=2, k=KP) \
                            .to_broadcast([128, nt, KP, 16, 2])
                        x_b = x_sup[:, ta * D:(ta + nt) * D] \
                            .rearrange("p (t o j2 r) -> p t o j2 r",
                                       o=1, r=2, j2=16) \
                            .to_broadcast([128, nt, KP, 16, 2])
                        z_ap = z_sb[:, zoff * ZW:(zoff + nt) * ZW] \
                            .rearrange("p (t k j2 r) -> p t k j2 r",
                                       k=KP, j2=16, r=2)
                        eng.tensor_tensor(out=z_ap, in0=x_b, in1=ea_b,
                                          op=mybir.AluOpType.mult)

                    # offload tail tiles of each group to gpsimd
                    zop(nc.vector, t0, gt - npool, 0)
                    zop(nc.gpsimd, t0 + gt - npool, npool, gt - npool)

                    u_ps = pu_pool.tile([128, gt * UW], F32, tag="u")
                    for tt in range(gt):
                        for g, (c0, w) in enumerate(grp):
                            col = tt * UW + g * TILE_S
                            zc = tt * ZW + c0
                            nc.tensor.matmul(
                                out=u_ps[0:w, col:col + TILE_S],
                                lhsT=z_sb[:, zc:zc + w],
                                rhs=a_sup[:, (t0 + tt) * TILE_S:
                                          (t0 + tt + 1) * TILE_S],
                                start=True, stop=True)

                    u_sb = upool.tile([128, gt * UW], F16, tag="usb")
                    nc.scalar.copy(out=u_sb[:], in_=u_ps[:])

                    for tt in range(gt):
                        for g, (c0, w) in enumerate(grp):
                            col = tt * UW + g * TILE_S
                            oc = (t0 - half * H + tt) * D
                            nc.tensor.matmul(
                                out=o_ps[:, oc:oc + D],
                                lhsT=u_sb[0:w, col:col + TILE_S],
                                rhs=b_sb[0:w, g * D:(g + 1) * D],
                                start=(g == 0), stop=(g == NG - 1))

                nc.scalar.copy(
                    out=out_sb[:, half * H * D:(half + 1) * H * D],
                    in_=o_ps[:])
                nc.sync.dma_start(
                    pout_d[s][:, half * H * D:(half + 1) * H * D],
                    out_sb[:, half * H * D:(half + 1) * H * D])

    nc.compile()
    return nc


_CACHE = {}


def kernel(node_attr, edge_attr, pair_indices, kernel, bias):
    per_core, meta = _prepare(node_attr, edge_attr, pair_indices,
                              kernel, bias)
    key = (meta["NSUP"], meta["KP"])
    if key not in _CACHE:
        _CACHE[key] = _build(*key)
    nc = _CACHE[key]
    res = run_bass_kernel_spmd(nc, per_core, list(range(NCORES)))
    pout = [res.results[c]["pout"] for c in range(NCORES)]
    return _unscatter(pout, meta["sidx"], meta["NSUP"])
